# revision 27
# baseline (speedup 1.0000x reference)
"""Trainium2 Bass kernel for batched attention with query-axis softmax.

Reference computation (per example b of 64):
    Q = q @ Wq.T + bq              # [S=1024, Y=128]
    K = q @ Wk.T + bk
    V = q @ Wv.T + bv
    scores = Q @ K.T / sqrt(Y)     # [Sq, Sk]
    attn   = softmax(scores, axis=-2)   # normalize over the QUERY axis
    out    = attn @ V              # [S, Y]
    result = max(out, axis=-2)     # [Y]

Key structural facts exploited here:
  * softmax normalizes over q, which is NOT the contraction axis of attn@V:
    out[q,d] = sum_k U[q,k]/c[k] * V[k,d] with U = exp(scores),
    c[k] = sum_q U[q,k].  So the normalization folds into V's rows:
    out = U @ (V / c).  No SxS division needed.
  * storing scores transposed (scoresT[k,q]) makes c a free-dim row-sum,
    which the ScalarE Exp instruction produces for free via accum_out.
  * outT[d,q] = V'.T-accumulated matmul keeps the final max a free-dim
    reduce_max -> [128,1] per example.
  * V is computed per k-tile directly in [k, d] layout (lhsT = a q
    chunk), with the bv bias added from a broadcast block during the
    PSUM drain; V's 1/c normalization folds into its rows.

Schedule notes:
  * w goes first on the Sync HWDGE ring (it gates every matmul), then
    example-0 q in two xb-major 256KB chunks; biases ride the Scalar
    ring in parallel.  The example-0 projections run xb-outer with
    per-half drains so matmuls start on first-chunk arrival and the
    first scores issues before the projection fully completes.
  * a dummy exp preloads the ACT exp table and a burst of dummy matmuls
    warms the PE HAM clock gate while the inputs stream in.
  * attnV accumulates into two separate 1-bank PSUM tiles (one per
    512-column half); the per-half reduce_max emissions are deferred by
    one step so they queue behind the next step's V-drain chain on the
    in-order DVE (a bunched reduce at the DVE head stalls the single V
    psum bank and with it the whole PE stream).
  * drain lag tapers to 2 on the last example; results store 6/2 so
    only two rows remain for the epilogue.

All matmul operands are fp16 (inputs rounded on host); accumulation is
always fp32 in PSUM and the softmax sums/normalization are fp32.

Sharding: data-parallel over batch, 8 examples per NeuronCore x 8 cores.
"""

import numpy as np
from contextlib import ExitStack

import concourse.bacc as bacc
import concourse.tile as tile
import concourse.mybir as mybir
import concourse.bass_utils as bass_utils

F32 = mybir.dt.float32
BF16 = mybir.dt.float16  # 16-bit matmul dtype: fp16 (11-bit significand)

NCORES = 8
B_PER_CORE = 8
S = 1024          # sequence length
X = 256           # input dim
Y = 128           # head dim
P = 128           # partitions
NH = 2            # 512-column halves of S (psum bank limit)
NKT = S // P      # 8 k-tiles
WARM_MMS = 30     # dummy matmuls bridge the PE from preamble end to data arrival


def emit(ctx, tc, out_d, ins):
    nc = tc.nc
    AF = mybir.ActivationFunctionType
    AX = mybir.AxisListType

    qt_d, w_d, b_d = ins

    wpool = ctx.enter_context(tc.tile_pool(name="w", bufs=1))
    qtp = ctx.enter_context(tc.tile_pool(name="qtp", bufs=4))
    qkp = ctx.enter_context(tc.tile_pool(name="qk", bufs=2))
    up = ctx.enter_context(tc.tile_pool(name="u", bufs=11))
    vrp = ctx.enter_context(tc.tile_pool(name="vr", bufs=4))
    vsp = ctx.enter_context(tc.tile_pool(name="vs", bufs=11))
    crp = ctx.enter_context(tc.tile_pool(name="cr", bufs=12))
    resp = ctx.enter_context(tc.tile_pool(name="res", bufs=1))
    # PSUM budget (8 banks): scores 2x2 + proj 1 + attnV accum 1+1 + V 1
    pmm = ctx.enter_context(tc.tile_pool(name="pmm", bufs=2, space="PSUM"))
    pprj = ctx.enter_context(tc.tile_pool(name="pprj", bufs=1, space="PSUM"))
    pout0 = ctx.enter_context(tc.tile_pool(name="pout0", bufs=1, space="PSUM"))
    pout1 = ctx.enter_context(tc.tile_pool(name="pout1", bufs=1, space="PSUM"))
    pvp = ctx.enter_context(tc.tile_pool(name="pv", bufs=1, space="PSUM"))

    # w first on the Sync ring (gates all compute), then example-0 q in two
    # xb-major 256KB chunks (2KB dram rows).  Biases on the Scalar ring.
    # w: [128, 7*128] bf16 -- wq | wk | wv ([128, 2*Y] each, x-chunk xb at
    #    cols xb*Y..; proj scale folded into wq) | bv broadcast block.
    # b: [128, 2+128] f32 -- bq_scaled | bk | identity (final transpose)
    w = wpool.tile([P, 7 * Y], BF16)
    nc.sync.dma_start(w[:, 0:4 * Y], w_d[:, 0:4 * Y])   # wq|wk: gates proj
    qt0 = qtp.tile([P, 2 * S], BF16, tag="qt")
    nc.sync.dma_start(qt0[:, 0:512], qt_d[0][0:P, 0:512])
    nc.sync.dma_start(qt0[:, 512:S], qt_d[0][0:P, 512:S])
    nc.sync.dma_start(qt0[:, S:2 * S], qt_d[0][P:2 * P, :])
    nc.scalar.dma_start(w[:, 4 * Y:7 * Y], w_d[:, 4 * Y:7 * Y])  # wv|bv
    bqk = wpool.tile([P, 2 + P], F32)
    nc.scalar.dma_start(bqk[:], b_d[:])
    wq = w[:, 0 * Y: 2 * Y]
    wk = w[:, 2 * Y: 4 * Y]
    wv = w[:, 4 * Y: 6 * Y]

    # ACT exp-table preload + PE HAM warm-up on scratch data, overlapping
    # the input DMAs. Outputs are never read.
    scr = wpool.tile([P, 128], BF16)
    nc.gpsimd.memset(scr[:], 0.0)
    scro = wpool.tile([P, 8], F32)
    nc.scalar.activation(scro[:], scr[:, 0:8], AF.Exp)
    pwarm = pprj.tile([P, 512], F32, tag="pj", name="pwarm")
    for _ in range(WARM_MMS):
        nc.tensor.matmul(pwarm[0:64, 0:128], lhsT=scr[:, 0:64],
                         rhs=scr[:, 0:128], start=True, stop=True)

    def load_qt(b):
        # qT[b] : [256, 1024] -> sbuf [128, 2*1024], x-chunk xb at cols xb*S..
        qt = qtp.tile([P, 2 * S], BF16, tag="qt")
        qv = qt_d[b].rearrange("(xb p) s -> p xb s", p=P)
        nc.sync.dma_start(qt[:].rearrange("p (xb s) -> p xb s", xb=2), qv)
        return qt

    def proj_mms(qt, w_sb, nh):
        # One 512-column half of a Q/K projection: ZT[y, s_half] = W.T @ qT
        pm = pprj.tile([P, 512], F32, tag="pj")
        for xb in range(2):
            nc.tensor.matmul(
                pm[:],
                lhsT=w_sb[:, xb * Y:(xb + 1) * Y],
                rhs=qt[:, xb * S + nh * 512: xb * S + nh * 512 + 512],
                start=(xb == 0),
                stop=(xb == 1),
            )
        return pm

    def proj_drain(pm, dst, bcol, nh):
        # psum -> sbuf with per-partition bias
        nc.vector.tensor_scalar_add(
            dst[:, nh * 512:(nh + 1) * 512], pm[:], bqk[:, bcol:bcol + 1]
        )

    def front(qt, QT, KT, kt, split_exp=False):
        """scores -> exp(+colsum) -> V -> V/c for one k-tile; returns (u, vs)."""
        # scoresT[k_tile, q] = KT_chunk.T @ QT   (contract d)
        ps = pmm.tile([P, S], F32, tag="mm")
        with tc.high_priority(offset=40):
            for nh in range(NH):
                nc.tensor.matmul(
                    ps[:, nh * 512:(nh + 1) * 512],
                    lhsT=KT[:, kt * P:(kt + 1) * P],
                    rhs=QT[:, nh * 512: nh * 512 + 512],
                    start=True,
                    stop=True,
                )

            # U = exp(scoresT), c[k] = sum_q U (free accumulation on ACT)
            u = up.tile([P, S], BF16, tag="u")
            c = crp.tile([P, 1], F32, tag="c")
            if split_exp:
                # last step: per-half exps pipeline with the scores matmuls,
                # shortening the kernel's closing exp->vs->attnV chain
                c0 = crp.tile([P, 1], F32, tag="ch", name="c0")
                nc.scalar.activation(u[:, 0:512], ps[:, 0:512], AF.Exp,
                                     accum_out=c0[:])
                c1 = crp.tile([P, 1], F32, tag="ch", name="c1")
                nc.scalar.activation(u[:, 512:S], ps[:, 512:S], AF.Exp,
                                     accum_out=c1[:])
                nc.vector.tensor_add(c[:], c0[:], c1[:])
            else:
                nc.scalar.activation(u[:], ps[:], AF.Exp, accum_out=c[:])

        # V k-tile directly in [k, d] layout: V[s_tile,:] = qT_chunk.T @ WvT
        pv = pvp.tile([P, P], F32, tag="pv")
        for xb in range(2):
            nc.tensor.matmul(
                pv[:],
                lhsT=qt[:, xb * S + kt * P: xb * S + (kt + 1) * P],
                rhs=wv[:, xb * Y:(xb + 1) * Y],
                start=(xb == 0),
                stop=(xb == 1),
            )
        # Drain V out of PSUM right away (frees the single pv bank without
        # waiting for c), adding the bv bias via partition-broadcast.
        vraw = vrp.tile([P, P], BF16, tag="vr")
        nc.vector.tensor_add(vraw[:], pv[:], w[:, 6 * Y:7 * Y])

        # V'[k, :] = V[k, :] / c[k]
        r = crp.tile([P, 1], F32, tag="r")
        nc.vector.reciprocal(r[:], c[:])
        vs = vsp.tile([P, P], BF16, tag="vs")
        nc.vector.tensor_scalar_mul(vs[:], vraw[:], r[:])
        return u, vs

    # Software-pipelined emission over a flat (b, kt) step stream.  The
    # attnV accumulation runs LAG steps behind the scores->exp front, and
    # example b+1's DMA + projections are emitted inside example b's k-loop.
    LAG = 5
    steps = [(b, kt) for b in range(B_PER_CORE) for kt in range(NKT)]
    state = {}       # b -> (qt, QT, KT)
    fifo = {}        # step index -> (b, kt, u, vs)
    po = [None, None]
    pending = []     # deferred per-example (b, po0, po1) reduce emissions

    res_all = resp.tile([P, B_PER_CORE], F32, tag="res")
    res_t1 = resp.tile([6, P], F32, tag="rest1")
    res_t2 = resp.tile([2, P], F32, tag="rest2")

    def transpose_store(cols, pt_name, res_t, out_rows):
        pt = pvp.tile([P, P], F32, tag="pv", name=pt_name)
        n = cols.stop - cols.start
        nc.tensor.transpose(pt[0:n, :], res_all[:, cols], bqk[:, 2:2 + P])
        nc.vector.tensor_copy(res_t[:], pt[0:n, :])
        nc.sync.dma_start(out_d[out_rows], res_t[:])

    def drain(i):
        b, kt, u, vs = fifo.pop(i)
        if kt == 0:
            po[0] = pout0.tile([P, 512], F32, tag="o0", name="po0")
            po[1] = pout1.tile([P, 512], F32, tag="o1", name="po1")
        # outT[d, q] += V'.T @ U   (contract k)
        for nh in range(NH):
            nc.tensor.matmul(
                po[nh][:],
                lhsT=vs[:],
                rhs=u[:, nh * 512: nh * 512 + 512],
                start=(kt == 0),
                stop=(kt == NKT - 1),
            )
        if kt == NKT - 1:
            pending.append((b, po[0], po[1]))

    def flush_reduces():
        while pending:
            b, q0, q1 = pending.pop(0)
            rh0 = crp.tile([P, 1], F32, tag="h0", bufs=2, name="rh0")
            nc.vector.reduce_max(rh0[:], q0[:], axis=AX.X)
            rh1 = crp.tile([P, 1], F32, tag="h1", bufs=2, name="rh1")
            nc.vector.reduce_max(rh1[:], q1[:], axis=AX.X)
            nc.vector.tensor_max(res_all[:, b:b + 1], rh0[:], rh1[:])
            if b == 5:
                # examples 0..5 done: transpose+store them now; only rows
                # 6..7 remain for the epilogue.
                transpose_store(slice(0, 6), "pt1", res_t1, slice(0, 6))

    # Prologue: example-0 Q/K projections, xb-outer through the idle scores
    # psum banks, per-half drains right after each xb1 matmul so the first
    # scores can issue before the full projection completes.
    QT0 = qkp.tile([P, S], BF16, tag="QT")
    KT0 = qkp.tile([P, S], BF16, tag="KT")
    pmQ = pmm.tile([P, S], F32, tag="mm", name="pmQ")
    pmK = pmm.tile([P, S], F32, tag="mm", name="pmK")
    for xb in range(2):
        for nh in range(NH):
            for w_sb, pm, dst, bcol in ((wq, pmQ, QT0, 0), (wk, pmK, KT0, 1)):
                nc.tensor.matmul(
                    pm[:, nh * 512:(nh + 1) * 512],
                    lhsT=w_sb[:, xb * Y:(xb + 1) * Y],
                    rhs=qt0[:, xb * S + nh * 512: xb * S + nh * 512 + 512],
                    start=(xb == 0),
                    stop=(xb == 1),
                )
                if xb == 1:
                    nc.vector.tensor_scalar_add(
                        dst[:, nh * 512:(nh + 1) * 512],
                        pm[:, nh * 512:(nh + 1) * 512],
                        bqk[:, bcol:bcol + 1],
                    )
    state[0] = (qt0, QT0, KT0)

    qtiles = {0: qt0}
    if B_PER_CORE > 1:
        qtiles[1] = load_qt(1)

    # per-example proj schedule: half j at step kt=j+1 (Q0 Q1 K0 K1)
    PROJ_PLAN = ((0, wq, 0), (0, wq, 0), (1, wk, 1), (1, wk, 1))

    for i, (b, kt) in enumerate(steps):
        qt, QT, KT = state[b]
        if kt == 0 and b + 2 < B_PER_CORE:
            qtiles[b + 2] = load_qt(b + 2)
        if kt == 0 and b + 1 < B_PER_CORE:
            state[b + 1] = (qtiles[b + 1],)
        if kt == 2 and b + 1 < B_PER_CORE:
            # allocate next example's projection outputs; halves fill in
            # one per step over kt=2..5
            QT_n = qkp.tile([P, S], BF16, tag="QT")
            KT_n = qkp.tile([P, S], BF16, tag="KT")
            state[b + 1] = (state[b + 1][0], QT_n, KT_n)
        prj = None
        if 2 <= kt <= 5 and b + 1 < B_PER_CORE:
            qt_n, QT_n, KT_n = state[b + 1]
            which, w_sb, bcol = PROJ_PLAN[kt - 2]
            dst = (QT_n, KT_n)[which]
            nh = (kt - 2) % 2
            pm = proj_mms(qt_n, w_sb, nh)
            prj = (pm, dst, bcol, nh)
        u, vs = front(qt, QT, KT, kt,
                      split_exp=(b == B_PER_CORE - 1 and kt == NKT - 1))
        fifo[i] = (b, kt, u, vs)
        flush_reduces()
        target = i - LAG
        if b == B_PER_CORE - 1 and kt >= 2:
            # taper on the last example down to two leftover drains; lag
            # stays >=2 so drains never chase a just-issued exp
            target = min(i - 2, i - 6 + kt)
        while fifo and min(fifo) <= target:
            drain(min(fifo))
        if prj is not None:
            # proj psum drain last: DVE order per step is the V-drain chain
            # first, deferred reduces, then this (DVE is in-order; a drain
            # at the head of the queue stalls the V psum bank recycle)
            proj_drain(*prj)
    for i in sorted(fifo):
        drain(i)
    flush_reduces()

    # Transpose the last two collected results and store them.
    transpose_store(slice(6, 8), "pt2", res_t2, slice(6, 8))


def build_program():
    nc = bacc.Bacc(
        "TRN2",
        target_bir_lowering=False,
        debug=False,
        enable_asserts=False,
    )
    qt = nc.dram_tensor("qt", [B_PER_CORE, X, S], BF16, kind="ExternalInput").ap()
    w = nc.dram_tensor("w", [P, 7 * Y], BF16, kind="ExternalInput").ap()
    b = nc.dram_tensor("b", [P, 2 + P], F32, kind="ExternalInput").ap()
    out = nc.dram_tensor("out", [B_PER_CORE, Y], F32, kind="ExternalOutput").ap()

    ins = (qt, w, b)
    with tile.TileContext(nc) as tc:
        with ExitStack() as ctx:
            emit(ctx, tc, out, ins)
    nc.compile()
    return nc


_NC_CACHE = None


def _get_program():
    global _NC_CACHE
    if _NC_CACHE is None:
        _NC_CACHE = build_program()
    return _NC_CACHE


def prep_inputs(q, Wq, bq, Wk, bk, Wv, bv):
    """Host-side marshalling: transpose q, pack weights, fold softmax scale."""
    q = np.asarray(q, dtype=np.float32)
    scale = np.float32(1.0 / np.sqrt(Y))
    f16 = np.float16

    qT = np.ascontiguousarray(q.transpose(0, 2, 1)).astype(f16)  # [B, X, S]

    def pack(w):  # [Y, X] torch layout -> [128, 2*Y]: chunk xb at cols xb*Y..
        wt = np.asarray(w, dtype=np.float32).T  # [X, Y]
        return np.concatenate([wt[0:P], wt[P:2 * P]], axis=1)

    w_all = np.concatenate(
        [pack(Wq) * scale, pack(Wk), pack(Wv),
         np.tile(np.asarray(bv, np.float32).reshape(1, Y), (P, 1))], axis=1
    ).astype(f16)
    b_all = np.concatenate(
        [np.stack([np.asarray(bq, np.float32) * scale,
                   np.asarray(bk, np.float32)], axis=1),
         np.eye(P, dtype=np.float32)], axis=1
    ).astype(np.float32)
    feeds = {
        "w": np.ascontiguousarray(w_all),
        "b": np.ascontiguousarray(b_all),
    }
    return qT, feeds


def kernel(q, Wq, bq, Wk, bk, Wv, bv, _trace=False):
    qT, feeds = prep_inputs(q, Wq, bq, Wk, bk, Wv, bv)
    nc = _get_program()
    in_maps = [
        {"qt": qT[c * B_PER_CORE:(c + 1) * B_PER_CORE], **feeds}
        for c in range(NCORES)
    ]
    kw = {}
    if _trace:
        kw = dict(trace=True)
    res = bass_utils.run_bass_kernel_spmd(
        nc, in_maps, core_ids=list(range(NCORES)), **kw
    )
    out = np.concatenate([r["out"] for r in res.results], axis=0)
    if _trace:
        return out, res
    return out
